# revision 38
# baseline (speedup 1.0000x reference)
"""BIDE forward kernel for Trainium2, 8-core data parallel over B — v3.

Two parallel pipelines per batch row (2 rows per core):

logZ path (enumerates all 2^16 patterns as a 256x256 table):
  table[hi, lo] = sum_h cos(zhi)cos(zlo) - sin(zhi)sin(zlo)  (two K=128
  matmuls over trig tables), logZ = 60 + ln(sum exp(table - 60)).

logit_x path (direct, no gather): q[h, t] = sum_n W'[h,n] bit_n(x_t) + r'
  as a K=17 matmul over host-precomputed bit-planes of x, then
  logit_x[t] = sum_h cos(2*pi*q[h,t]) via Sin + one-hot-column matmuls
  that fold h into a [16, 512] PSUM tile already shaped like the output.

Range reduction (Sin only accepts [-pi, pi]; DVE has no mod op and
rint+subtract costs two DVE passes): the matmul's constant row adds
cp + 192 so PSUM holds q' = q + cp + 192 in [128, 256), where bf16's ULP
is exactly 1 — a single DVE f32->bf16 copy rounds q' to the nearest
INTEGER. A second PE matmul accumulates -I @ round(q') into the same
PSUM bank, leaving w = centered-frac(q + cp), and Sin(2*pi*w) needs no
bias. One DVE pass + free PE work instead of two DVE passes.

Per-element gathers are unusable on this part: indirect-DMA descriptors
retire at ~4.5ns each *serialized* (measured: completion semaphores lag
data by 20-30us for 8k descriptors), and gpsimd ap_gather ucode runs
~27us per 512 indices. Hence the direct logit_x computation.
"""

import numpy as np
import ml_dtypes
from contextlib import ExitStack

import concourse.bacc as bacc
import concourse.bass as bass
from concourse import mybir
from concourse.bass_utils import run_bass_kernel_spmd
from concourse.tile import TileContext

F32 = mybir.dt.float32
BF16 = mybir.dt.bfloat16
I32 = mybir.dt.int32

TWO_PI = float(np.float32(2.0 * np.pi))
INV_2PI = 1.0 / (2.0 * np.pi)
# logits peak ~89: shift exp so it stays in fp32 / the ACT Ln spline range
EXP_SHIFT = 60.0
# q + MAGIC lands in [128, 256) where bf16 ULP = 1, so bf16-rounding = rint
MAGIC = 192.0

N_CORES = 8
B, H, T = 16, 128, 4096
BPC = B // N_CORES  # batch rows per core (2)


def _build():
    nc = bacc.Bacc("TRN2", target_bir_lowering=False, debug=False)

    # table-path z weights: quarter g of row b at cols 128*(4b+g), quarters
    # = (cos-lo, cos-hi, sin-lo, sin-hi); rows 0-7 W'bits, 8 r'(hi), 9 magic
    wp = nc.dram_tensor("wp", [10, 1024], BF16, kind="ExternalInput")
    # bit-plane enumeration of [0,256): rows 0-7 = (v>>k)&1, rows 8-9 = 1
    bits = nc.dram_tensor("bits", [10, 256], BF16, kind="ExternalInput")
    # x-path z weights: rows 0-15 = W'[b].T, row 16 = r' + 0.25, row 17 =
    # MAGIC (kept separate: bf16 ULP at 192 is 1.0 and would wipe out r')
    wx = nc.dram_tensor("wx", [18, 256], BF16, kind="ExternalInput")
    # bit-planes of x: row n = bit_n(x[b, t]), rows 16-17 = 1; row b at 4096b
    bitsx = nc.dram_tensor("bitsx", [18, 8192], BF16, kind="ExternalInput")
    negi = nc.dram_tensor("negi", [128, 128], BF16, kind="ExternalInput")
    # h-sum one-hot columns: hsw[h, 16*v + m] = (m == v), v = 8b + t//512
    hsw = nc.dram_tensor("hsw", [128, 256], BF16, kind="ExternalInput")
    # negsel[k, m] = -1 if m//8 == k else 0 (broadcasts -ln S_b)
    negsel_in = nc.dram_tensor("negsel", [2, 16], F32, kind="ExternalInput")
    out = nc.dram_tensor("out", [BPC, T], F32, kind="ExternalOutput")
    dbg_tq = nc.dram_tensor("dbg_tq", [128, 512], BF16, kind="ExternalOutput")
    dbg_w = nc.dram_tensor("dbg_w", [128, 512], F32, kind="ExternalOutput")
    dbg_sx = nc.dram_tensor("dbg_sx", [128, 4096], BF16, kind="ExternalOutput")
    dbg_tc = nc.dram_tensor("dbg_tc", [128, 512], BF16, kind="ExternalOutput")
    dbg_ts = nc.dram_tensor("dbg_ts", [128, 512], BF16, kind="ExternalOutput")
    dbg_tp = nc.dram_tensor("dbg_tp", [128, 512], F32, kind="ExternalOutput")
    dbg_lz = nc.dram_tensor("dbg_lz", [2, 1], F32, kind="ExternalOutput")

    with ExitStack() as ctx:
        tc = ctx.enter_context(TileContext(nc))
        sb = ctx.enter_context(tc.tile_pool(name="sb", bufs=1))
        psa = ctx.enter_context(tc.tile_pool(name="psa", bufs=2, space="PSUM"))
        psb = ctx.enter_context(tc.tile_pool(name="psb", bufs=2, space="PSUM"))
        pst = ctx.enter_context(tc.tile_pool(name="pst", bufs=2, space="PSUM"))
        psh = ctx.enter_context(tc.tile_pool(name="psh", bufs=1, space="PSUM"))
        pss = ctx.enter_context(tc.tile_pool(name="pss", bufs=1, space="PSUM"))

        # ---- input loads
        bitsx_sb = sb.tile([18, 8192], BF16, tag="bitsx")
        wx_sb = sb.tile([18, 256], BF16, tag="wx")
        wp_sb = sb.tile([10, 1024], BF16, tag="wp")
        bits_sb = sb.tile([10, 256], BF16, tag="bits")
        negi_sb = sb.tile([128, 128], BF16, tag="negi")
        hsw_sb = sb.tile([128, 256], BF16, tag="hsw")
        negsel = sb.tile([2, 16], F32, tag="negsel")
        nc.sync.dma_start(out=bitsx_sb[:], in_=bitsx[:])
        nc.sync.dma_start(out=wx_sb[:], in_=wx[:])
        nc.sync.dma_start(out=wp_sb[:], in_=wp[:])
        nc.sync.dma_start(out=bits_sb[:], in_=bits[:])
        nc.sync.dma_start(out=negi_sb[:], in_=negi[:])
        nc.sync.dma_start(out=hsw_sb[:], in_=hsw[:])
        nc.sync.dma_start(out=negsel[:], in_=negsel_in[:])

        # ---- constants
        ones = sb.tile([128, 1], F32, tag="ones")
        nc.vector.memset(ones[:], 1.0)

        sums2 = sb.tile([128, 2], F32, tag="sums2")
        hs_ps = psh.tile([16, 512], F32, tag="hs")
        tb_ps = []

        # A-region: closed q' group (legal for the DVE cast to read);
        # B-region: q' recomputed + (-I @ round(q')) accumulated -> w; the
        # ACT Sin reads B. Never read an open PSUM accumulation group.
        def q_unit(mk_qmms, tag, width=512):
            qa = psa.tile([128, 512], F32, tag="qa")
            mk_qmms(qa, True)
            tq = sb.tile([128, width], BF16, tag=tag)
            nc.vector.tensor_scalar(
                out=tq[:], in0=qa[:, 0:width], scalar1=0.0, scalar2=None,
                op0=mybir.AluOpType.add,
            )
            q_unit.last_tq = tq
            qb = psb.tile([128, 512], F32, tag="qb")
            mk_qmms(qb, False)
            nc.tensor.matmul(
                out=qb[:, 0:width], lhsT=negi_sb[:], rhs=tq[:],
                start=False, stop=True,
            )
            return qb

        for b in range(BPC):
            # ---- table path: one unit per quarter (one open accumulation
            # group per PSUM bank at a time — two opens in one bank corrupt)
            tcos = sb.tile([128, 512], BF16, tag=f"tc{b}")
            tsin = sb.tile([128, 512], BF16, tag=f"ts{b}")
            for g in range(4):
                def tbl_qmm(dst, closed, g=g):
                    nc.tensor.matmul(
                        out=dst[:, 0:256],
                        lhsT=wp_sb[:, 128 * (4 * b + g) : 128 * (4 * b + g) + 128],
                        rhs=bits_sb[:],
                        start=True, stop=closed,
                    )

                qbg = q_unit(tbl_qmm, f"tqu{b}{g}", width=256)
                dst = tcos if g < 2 else tsin
                nc.scalar.activation(
                    out=dst[:, 256 * (g % 2) : 256 * (g % 2) + 256],
                    in_=qbg[:, 0:256],
                    func=mybir.ActivationFunctionType.Sin,
                    scale=TWO_PI if g < 3 else -TWO_PI,
                )
            # table[hi, lo] = cos(zhi)cos(zlo) - sin(zhi)sin(zlo)
            tp = pst.tile([128, 512], F32, tag="tb")
            tb_ps.append(tp)
            for c in range(2):
                cs = slice(256 * c, 256 * c + 256)
                hi_s = slice(256 + 128 * c, 256 + 128 * c + 128)
                nc.tensor.matmul(
                    out=tp[:, cs], lhsT=tcos[:, hi_s], rhs=tcos[:, 0:256],
                    start=True, stop=False,
                )
                nc.tensor.matmul(
                    out=tp[:, cs], lhsT=tsin[:, hi_s], rhs=tsin[:, 0:256],
                    start=False, stop=True,
                )
            if b == 0:
                nc.sync.dma_start(out=dbg_tc[:], in_=tcos[:])
                nc.sync.dma_start(out=dbg_ts[:], in_=tsin[:])
                tpd = sb.tile([128, 512], F32, tag="tpd")
                nc.vector.tensor_copy(out=tpd[:], in_=tp[:])
                nc.sync.dma_start(out=dbg_tp[:], in_=tpd[:])

            # ---- x path: 8 chunks of 512 t's
            sxb = sb.tile([128, 4096], BF16, tag=f"sx{b}")
            for c in range(8):
                col = 4096 * b + 512 * c

                def x_qmm(dst, closed):
                    nc.tensor.matmul(
                        out=dst[:],
                        lhsT=wx_sb[:, 128 * b : 128 * b + 128],
                        rhs=bitsx_sb[:, col : col + 512],
                        start=True, stop=closed,
                    )

                qxb = q_unit(x_qmm, f"tbf{b}{c}")
                if b == 0 and c == 0:
                    nc.sync.dma_start(out=dbg_tq[:], in_=q_unit.last_tq[:])
                    wd = sb.tile([128, 512], F32, tag="wd")
                    nc.vector.tensor_copy(out=wd[:], in_=qxb[:])
                    nc.sync.dma_start(out=dbg_w[:], in_=wd[:])
                nc.scalar.activation(
                    out=sxb[:, 512 * c : 512 * c + 512], in_=qxb[:],
                    func=mybir.ActivationFunctionType.Sin, scale=TWO_PI,
                )
                # h-fold: hs[8b + c, j] += sum_h cos
                vg = 8 * b + c
                nc.tensor.matmul(
                    out=hs_ps[:],
                    lhsT=hsw_sb[:, 16 * vg : 16 * vg + 16],
                    rhs=sxb[:, 512 * c : 512 * c + 512],
                    start=(vg == 0), stop=(vg == 15),
                )
            if b == 0:
                nc.sync.dma_start(out=dbg_sx[:], in_=sxb[:])

        # ---- logZ: exp with accumulator row-sums. The exp bias tile is
        # DERIVED from the last Sin's output (x0 = column of sxb times 0) so
        # the Exps cannot be scheduled between Sins on the ACT engine (each
        # such slot costs a 1.5us activation-table reload).
        neg_shift2 = sb.tile([128, 1], F32, tag="neg_shift2")
        nc.vector.tensor_scalar(
            out=neg_shift2[:], in0=sxb[:, 4095:4096], scalar1=0.0,
            scalar2=-EXP_SHIFT, op0=mybir.AluOpType.mult, op1=mybir.AluOpType.add,
        )
        e_sb = sb.tile([128, 1024], BF16, tag="e")
        for b in range(BPC):
            nc.scalar.activation(
                out=e_sb[:, 512 * b : 512 * b + 512], in_=tb_ps[b][:],
                func=mybir.ActivationFunctionType.Exp,
                bias=neg_shift2[:],
                accum_out=sums2[:, b : b + 1],
            )
        small_ps = pss.tile([16, 1], F32, tag="small")
        nc.tensor.matmul(
            out=small_ps[0:2, 0:1], lhsT=sums2[:], rhs=ones[:], start=True, stop=True
        )
        logz2 = sb.tile([2, 1], F32, tag="logz2")
        nc.scalar.activation(
            out=logz2[:], in_=small_ps[0:2, 0:1],
            func=mybir.ActivationFunctionType.Ln,
        )
        nc.sync.dma_start(out=dbg_lz[:], in_=logz2[:])
        # broadcast -ln(S_b) to the 16 output partitions (reuses the bank)
        nz_ps = small_ps
        nc.tensor.matmul(out=nz_ps[:], lhsT=negsel[:], rhs=logz2[:], start=True, stop=True)
        nz_sb = sb.tile([16, 1], F32, tag="nzsb")
        nc.vector.tensor_scalar(
            out=nz_sb[:], in0=nz_ps[:], scalar1=-EXP_SHIFT, scalar2=None,
            op0=mybir.AluOpType.add,
        )

        # ---- out[b, t] = logit_x - logZ_b
        o_t = sb.tile([16, 512], F32, tag="o")
        nc.vector.tensor_scalar(
            out=o_t[:], in0=hs_ps[:], scalar1=nz_sb[:], scalar2=None,
            op0=mybir.AluOpType.add,
        )
        for b in range(BPC):
            nc.sync.dma_start(
                out=out[b, :].rearrange("(c j) -> c j", c=8),
                in_=o_t[8 * b : 8 * b + 8, :],
            )


    nc.finalize()
    return nc


_NC = None


def _get_nc():
    global _NC
    if _NC is None:
        _NC = _build()
    return _NC


def _make_in_maps(x, W, r):
    x = np.asarray(x, dtype=np.int32)
    W = np.asarray(W, dtype=np.float32)
    r = np.asarray(r, dtype=np.float32)

    v = np.arange(256, dtype=np.int32)
    k8 = np.arange(8, dtype=np.int32)
    bp8 = ((v[None, :] >> k8[:, None]) & 1).astype(np.float32)  # [8, 256]
    bits = np.ones((10, 256), dtype=np.float32)
    bits[0:8] = bp8
    bits = bits.astype(ml_dtypes.bfloat16)

    k16 = np.arange(16, dtype=np.int32)
    negi = (-np.eye(128, dtype=np.float32)).astype(ml_dtypes.bfloat16)
    hsw = np.zeros((128, 256), dtype=np.float32)
    for vg in range(16):
        hsw[:, 16 * vg + vg] = 1.0
    hsw = hsw.astype(ml_dtypes.bfloat16)
    negsel = np.zeros((2, 16), dtype=np.float32)
    negsel[0, 0:8] = -1.0
    negsel[1, 8:16] = -1.0

    in_maps = []
    for core in range(N_CORES):
        wp = np.zeros((10, 1024), dtype=ml_dtypes.bfloat16)
        wxm = np.zeros((18, 256), dtype=ml_dtypes.bfloat16)
        bxs = []
        for b_loc in range(BPC):
            b = BPC * core + b_loc
            Wp = (W[b].T * INV_2PI).astype(ml_dtypes.bfloat16)  # [16, 128]
            rp = (r[b] * INV_2PI).astype(ml_dtypes.bfloat16).astype(np.float32)
            for g in range(4):
                # g: 0 coslo, 1 coshi, 2 sinlo, 3 sinhi
                half = 1 if g in (1, 3) else 0
                cp = 0.25 if g in (0, 1) else 0.0
                cs = slice(128 * (4 * b_loc + g), 128 * (4 * b_loc + g) + 128)
                wp[0:8, cs] = Wp[8 * half : 8 * half + 8]
                # r'+cp stays small (bf16-safe); MAGIC exact in its own row
                # (bf16 ULP at 192 is 1.0 -- adding r'+cp here would erase it)
                wp[8, cs] = ((rp if half else 0.0) + np.float32(cp)).astype(
                    ml_dtypes.bfloat16
                )
                wp[9, cs] = np.float32(MAGIC)
            xs = slice(128 * b_loc, 128 * b_loc + 128)
            wxm[0:16, xs] = Wp
            wxm[16, xs] = (rp + np.float32(0.25)).astype(ml_dtypes.bfloat16)
            wxm[17, xs] = np.float32(MAGIC)
            bx = np.ones((18, 4096), dtype=np.float32)
            bx[0:16] = ((x[b][None, :] >> k16[:, None]) & 1).astype(np.float32)
            bxs.append(bx.astype(ml_dtypes.bfloat16))
        in_maps.append(
            {
                "wp": wp,
                "bits": bits,
                "wx": wxm,
                "bitsx": np.concatenate(bxs, axis=1),
                "negi": negi,
                "hsw": hsw,
                "negsel": negsel,
            }
        )
    return in_maps


def _run(x, W, r, trace=False):
    nc = _get_nc()
    in_maps = _make_in_maps(x, W, r)
    res = run_bass_kernel_spmd(nc, in_maps, core_ids=list(range(N_CORES)), trace=trace)
    out = np.concatenate([res.results[c]["out"] for c in range(N_CORES)], axis=0)
    return out.astype(np.float32), res


def kernel(x, W, r):
    out, _ = _run(x, W, r)
    return out


def kernel_traced(x, W, r):
    out, res = _run(x, W, r, trace=True)
    return out, res
